# revision 1
# baseline (speedup 1.0000x reference)
"""Trainium2 Bass kernel for causal self-attention (nn_CausalSelfAttention).

Sharding: 8 cores = 4 batches x 2 head-groups (TP over heads).
Core c handles batch b=c//2, head-group g=c%2 (8 of 16 heads).
QKV column-parallel, c_proj row-parallel, AllReduce over pairs after c_proj.

Per-core device program (all matmuls float32r unless noted):
  per t-block (512 rows):
    QKV projection from host-pretransposed xT / wT
    RMS-norm + rotary on q,k in natural [t, h*d] layout (DVE/ACT)
    PE-transpose q,k -> d-major [d, t] (fp32, exact)
    lambda-mix v with ve, append ones column (softmax denominator trick)
    flash-style causal attention per head: S^T tiles [tk,128 x tq,512],
      exp on ACT, diagonal masking by multiplicative {0,1} masks,
      PV accumulate into [65, 512] PSUM (row 64 = sum of probs)
    normalize via reciprocal + ones-outer-product broadcast (fp32 matmul)
    c_proj partial, DMA to DRAM, AllReduce pair, write external output
"""

import numpy as np

import concourse.bass as bass
import concourse.mybir as mybir
import concourse.tile as tile
from concourse import bacc
from concourse.alu_op_type import AluOpType as OP
from concourse.bass_utils import run_bass_kernel_spmd
from concourse.masks import make_identity

F32 = mybir.dt.float32
F32R = mybir.dt.float32r
AFT = mybir.ActivationFunctionType

ATTN_SCALE = 0.12
EPS = 1e-6
D = 64  # head dim
NB_W = 512  # t-block width
N_CORES = 8


def build_nc(T, C, O, n_cores=N_CORES, loop_reps=1):
    """O = per-core output columns for each of q/k/v (= heads_per_core * 64)."""
    HLOC = O // D
    CT = C // 128
    TT = T // 128
    NB = T // NB_W
    SUB = NB_W // 128
    ODT = O // 128  # d-major tiles per projection
    OBW = min(512, C)
    OB = C // OBW

    nc = bacc.Bacc("TRN2", target_bir_lowering=False, debug=False,
                   num_devices=n_cores)

    xT = nc.dram_tensor("xT", [C, T], F32, kind="ExternalInput")
    wT = nc.dram_tensor("wT", [C, 3 * O], F32, kind="ExternalInput")
    cpT = nc.dram_tensor("cpT", [O, C], F32, kind="ExternalInput")
    vein = nc.dram_tensor("vein", [T, O], F32, kind="ExternalInput")
    cost = nc.dram_tensor("cost", [T, 32], F32, kind="ExternalInput")
    sint = nc.dram_tensor("sint", [T, 32], F32, kind="ExternalInput")
    mskt = nc.dram_tensor("mskt", [SUB, 128, NB_W], F32, kind="ExternalInput")
    lam0 = nc.dram_tensor("lam0", [128, 1], F32, kind="ExternalInput")
    lam1 = nc.dram_tensor("lam1", [128, 1], F32, kind="ExternalInput")
    out_e = nc.dram_tensor("out", [T, C], F32, kind="ExternalOutput")

    groups = [[2 * i, 2 * i + 1] for i in range(n_cores // 2)]

    with tile.TileContext(nc) as tc:
        with (
            tc.tile_pool(name="dram", bufs=1, space="DRAM") as dram,
            tc.tile_pool(name="const", bufs=1) as const,
            tc.tile_pool(name="resid", bufs=1) as resid,
            tc.tile_pool(name="xpool", bufs=2) as xpool,
            tc.tile_pool(name="vepool", bufs=2) as vepool,
            tc.tile_pool(name="natp", bufs=2) as natp,
            tc.tile_pool(name="small", bufs=2) as small,
            tc.tile_pool(name="qtp", bufs=2) as qtp,
            tc.tile_pool(name="ytp", bufs=1) as ytp,
            tc.tile_pool(name="ppool", bufs=2) as ppool,
            tc.tile_pool(name="opool", bufs=2) as opool,
            tc.tile_pool(name="qkv_ps", bufs=2, space="PSUM") as qkv_ps,
            tc.tile_pool(name="s_ps", bufs=2, space="PSUM") as s_ps,
            tc.tile_pool(name="y_ps", bufs=2, space="PSUM") as y_ps,
            tc.tile_pool(name="misc_ps", bufs=2, space="PSUM") as misc_ps,
        ):
            part = dram.tile([T, C], F32, name="part")
            arout = dram.tile([T, C], F32, name="arout")

            # ---- residents ----
            w_sb = resid.tile([128, CT, 3 * O], F32R, name="w_sb")
            nc.sync.dma_start(
                out=w_sb[:], in_=wT.ap().bitcast(F32R).rearrange("(ct p) o -> p ct o", p=128))
            cp_sb = resid.tile([128, ODT, C], F32R, name="cp_sb")
            nc.sync.dma_start(
                out=cp_sb[:], in_=cpT.ap().bitcast(F32R).rearrange("(ct p) o -> p ct o", p=128))
            cos_sb = resid.tile([128, TT, 32], F32, name="cos_sb")
            nc.sync.dma_start(
                out=cos_sb[:], in_=cost.ap().rearrange("(tt p) f -> p tt f", p=128))
            sin_sb = resid.tile([128, TT, 32], F32, name="sin_sb")
            nc.sync.dma_start(
                out=sin_sb[:], in_=sint.ap().rearrange("(tt p) f -> p tt f", p=128))
            mask_sb = resid.tile([128, SUB, NB_W], F32, name="mask_sb")
            nc.sync.dma_start(
                out=mask_sb[:], in_=mskt.ap().rearrange("m p f -> p m f"))
            lam0_sb = const.tile([128, 1], F32, name="lam0_sb")
            nc.sync.dma_start(out=lam0_sb[:], in_=lam0.ap())
            lam1_sb = const.tile([128, 1], F32, name="lam1_sb")
            nc.sync.dma_start(out=lam1_sb[:], in_=lam1.ap())
            ident = const.tile([128, 128], F32, name="ident")
            make_identity(nc, ident[:])
            ones_sb = const.tile([1, D], F32, name="ones_sb")
            nc.vector.memset(ones_sb[:], 1.0)
            eps_sb = const.tile([128, 1], F32, name="eps_sb")
            nc.vector.memset(eps_sb[:], EPS)
            one128 = const.tile([128, 1], F32, name="one128")
            nc.vector.memset(one128[:], 1.0)

            kT_tiles = []
            for i in range(NB):
                kT_tiles.append(
                    resid.tile([128, ODT, NB_W], F32R, name=f"kT{i}"))
            v_all = resid.tile([128, TT, HLOC, D + 1], F32R, name="v_all")
            nc.vector.tensor_copy(
                v_all[:, :, :, D:D + 1],
                one128[:, 0:1][:, None, None, :]
                .broadcast_to([128, TT, HLOC, 1]))

            xT_r = xT.ap().bitcast(F32R).rearrange("(ct p) t -> p ct t", p=128)

            import contextlib
            loop_cm = (tc.For_i(0, loop_reps, 1) if loop_reps > 1
                       else contextlib.nullcontext())
            with loop_cm:
              for i in range(NB):
                  qT_sb = qtp.tile([128, ODT, NB_W], F32R, tag="qT")
                  yt_sb = ytp.tile([128, ODT, NB_W], F32R, tag="yt")

                  # ---- QKV + norm + rotary + transpose ----
                  for s4 in range(SUB):
                      tt = i * SUB + s4
                      x_sb = xpool.tile([128, CT, 128], F32R, tag="x")
                      nc.sync.dma_start(
                          out=x_sb[:], in_=xT_r[:, :, tt * 128:(tt + 1) * 128])
                      ve_sb = vepool.tile([128, O], F32, tag="ve")
                      nc.sync.dma_start(
                          out=ve_sb[:], in_=vein.ap()[tt * 128:(tt + 1) * 128, :])

                      cb = cos_sb[:, tt, :][:, None, :].broadcast_to([128, HLOC, 32])
                      sb = sin_sb[:, tt, :][:, None, :].broadcast_to([128, HLOC, 32])

                      for proj in range(2):  # 0=q, 1=k
                          ps = qkv_ps.tile([128, O], F32, tag="qkv")
                          for ct in range(CT):
                              nc.tensor.matmul(
                                  ps[:],
                                  lhsT=x_sb[:, ct, :],
                                  rhs=w_sb[:, ct, proj * O:(proj + 1) * O],
                                  start=(ct == 0), stop=(ct == CT - 1))
                          psv = ps[:].rearrange("p (h d) -> p h d", d=D)
                          sqs = natp.tile([128, O], F32, tag="sqs")
                          nc.scalar.activation(sqs[:], ps[:], AFT.Square)
                          ssq = small.tile([128, HLOC], F32, tag="ssq")
                          nc.vector.reduce_sum(
                              ssq[:],
                              sqs[:].rearrange("p (h d) -> p h d", d=D),
                              axis=mybir.AxisListType.X)
                          rstd = small.tile([128, HLOC], F32, tag="rstd")
                          nc.scalar.activation(rstd[:], ssq[:], AFT.Sqrt,
                                               bias=eps_sb[:, 0:1], scale=1.0 / D)
                          rinv = small.tile([128, HLOC], F32, tag="rinv")
                          nc.vector.reciprocal(rinv[:], rstd[:])
                          qn = natp.tile([128, HLOC, D], F32, tag="qn")
                          nc.vector.tensor_tensor(
                              qn[:], psv[:],
                              rinv[:][:, :, None].broadcast_to([128, HLOC, D]),
                              op=OP.mult)
                          # rotary
                          x1 = qn[:, :, 0:32]
                          x2 = qn[:, :, 32:64]
                          qr = natp.tile([128, HLOC, D], F32, tag="qr")
                          rot = natp.tile([128, 4, HLOC, 32], F32, tag="rot")
                          ta, tb = rot[:, 0], rot[:, 1]
                          ua, ub = rot[:, 2], rot[:, 3]
                          nc.vector.tensor_tensor(ta, x2, sb, op=OP.mult)
                          nc.vector.tensor_tensor(tb, x2, cb, op=OP.mult)
                          nc.vector.tensor_tensor(ua, x1, sb, op=OP.mult)
                          nc.vector.tensor_tensor(ub, x1, cb, op=OP.mult)
                          nc.vector.tensor_tensor(qr[:, :, 0:32], ub, ta,
                                                  op=OP.add)
                          nc.vector.tensor_tensor(qr[:, :, 32:64], tb, ua,
                                                  op=OP.subtract)
                          qr2 = qr[:].rearrange("p h d -> p (h d)")
                          dst = qT_sb if proj == 0 else kT_tiles[i]
                          col0 = s4 * 128
                          for dt in range(ODT):
                              tp = misc_ps.tile([128, 128], F32, tag="m", name="tp")
                              nc.tensor.transpose(
                                  tp[:], qr2[:, dt * 128:(dt + 1) * 128], ident[:])
                              nc.scalar.copy(dst[:, dt, col0:col0 + 128], tp[:])

                      # v projection + lambda mix
                      ps = qkv_ps.tile([128, O], F32, tag="qkv")
                      for ct in range(CT):
                          nc.tensor.matmul(
                              ps[:],
                              lhsT=x_sb[:, ct, :],
                              rhs=w_sb[:, ct, 2 * O:3 * O],
                              start=(ct == 0), stop=(ct == CT - 1))
                      nc.vector.tensor_scalar_mul(ve_sb[:], ve_sb[:],
                                                  lam1_sb[:, 0:1])
                      nc.vector.scalar_tensor_tensor(
                          out=v_all[:, tt, :, 0:D],
                          in0=ps[:].rearrange("p (h d) -> p h d", d=D),
                          scalar=lam0_sb[:, 0:1],
                          in1=ve_sb[:].rearrange("p (h d) -> p h d", d=D),
                          op0=OP.mult, op1=OP.add)

                  # ---- attention for tq-block i ----
                  njt = i * SUB + SUB  # tk tiles 0..njt-1
                  for h in range(HLOC):
                      po = (h % 2) * D
                      dt = h // 2
                      yps = y_ps.tile([D + 1, NB_W], F32, tag="y")
                      for j in range(njt):
                          sps = s_ps.tile([128, NB_W], F32, tag="s")
                          nc.tensor.matmul(
                              sps[:],
                              lhsT=kT_tiles[j // SUB][po:po + D, dt,
                                                      (j % SUB) * 128:
                                                      (j % SUB) * 128 + 128]
                              ,
                              rhs=qT_sb[po:po + D, dt, :],
                              start=True, stop=True)
                          p_sb = ppool.tile([128, NB_W], F32R, tag="p")
                          nc.scalar.activation(p_sb[:], sps[:], AFT.Exp,
                                               bias=0.0, scale=ATTN_SCALE)
                          m = j - i * SUB
                          if m >= 0:
                              nc.vector.tensor_tensor(
                                  p_sb[:], p_sb[:], mask_sb[:, m, :], op=OP.mult)
                          nc.tensor.matmul(
                              yps[:],
                              lhsT=v_all[:, j, h, :],
                              rhs=p_sb[:],
                              start=(j == 0), stop=(j == njt - 1))
                      rec = opool.tile([1, NB_W], F32, tag="o", name="rec")
                      nc.vector.reciprocal(rec[:], yps[D:D + 1, :])
                      bc = misc_ps.tile([D, NB_W], F32, tag="m", name="bc")
                      nc.tensor.matmul(bc[:], lhsT=ones_sb[:], rhs=rec[:],
                                       start=True, stop=True)
                      bcs = opool.tile([D, NB_W], F32, tag="o", name="bcs")
                      nc.scalar.copy(bcs[:], bc[:])
                      nc.vector.tensor_tensor(
                          yt_sb[po:po + D, dt, :], yps[0:D, :], bcs[:], op=OP.mult)

                  # ---- c_proj partial for block i ----
                  for s4 in range(SUB):
                      r0 = (i * SUB + s4) * 128
                      for ob in range(OB):
                          cps = misc_ps.tile([128, OBW], F32, tag="m", name="cps")
                          for ct in range(ODT):
                              nc.tensor.matmul(
                                  cps[:],
                                  lhsT=yt_sb[:, ct, s4 * 128:(s4 + 1) * 128]
                                  ,
                                  rhs=cp_sb[:, ct, ob * OBW:(ob + 1) * OBW],
                                  start=(ct == 0), stop=(ct == ODT - 1))
                          o_sb = opool.tile([128, OBW], F32, tag="o")
                          nc.scalar.copy(o_sb[:], cps[:])
                          nc.sync.dma_start(
                              out=part[r0:r0 + 128, ob * OBW:(ob + 1) * OBW],
                              in_=o_sb[:])

                  # ---- AllReduce + output for block i ----
                  nc.gpsimd.collective_compute(
                      "AllReduce", OP.add, replica_groups=groups,
                      ins=[part[i * NB_W:(i + 1) * NB_W, :].opt()],
                      outs=[arout[i * NB_W:(i + 1) * NB_W, :].opt()])
                  nc.sync.dma_start(
                      out=out_e.ap()[i * NB_W:(i + 1) * NB_W, :],
                      in_=arout[i * NB_W:(i + 1) * NB_W, :])

    nc.compile()
    return nc


def make_tables(T, dtype=np.float32):
    nfreq = D // 4
    angular = (np.float32(1.0 / 1024.0)
               ** np.linspace(0.0, 1.0, nfreq, dtype=np.float32))
    angular = np.concatenate([angular, np.zeros(nfreq, np.float32)])
    t = np.arange(T, dtype=np.float32)
    theta = t[:, None] * angular[None, :]
    return np.cos(theta).astype(dtype), np.sin(theta).astype(dtype)


def make_masks():
    SUB = NB_W // 128
    masks = np.zeros((SUB, 128, NB_W), np.float32)
    for m in range(SUB):
        for r in range(128):
            masks[m, r, 128 * m + r:] = 1.0
    return masks


def prep_core_inputs(x, ve, qkv_w, lambdas, c_proj_w, core, n_groups=2):
    B = x.shape[0]
    T = x.shape[1]
    C = x.shape[2]
    O = C // n_groups
    b, g = core // n_groups, core % n_groups
    cols = slice(g * O, (g + 1) * O)
    xT = np.ascontiguousarray(x[b].T)
    wTm = np.concatenate(
        [np.ascontiguousarray(qkv_w[p, cols, :].T) for p in range(3)], axis=1)
    cpT = np.ascontiguousarray(c_proj_w[:, cols].T)
    cos_t, sin_t = make_tables(T)
    return {
        "xT": xT,
        "wT": np.ascontiguousarray(wTm),
        "cpT": cpT,
        "vein": np.ascontiguousarray(ve[b, :, cols]),
        "cost": cos_t,
        "sint": sin_t,
        "mskt": make_masks(),
        "lam0": np.full((128, 1), lambdas[0], np.float32),
        "lam1": np.full((128, 1), lambdas[1], np.float32),
    }


_NC_CACHE = {}


def get_nc(T, C, O):
    key = (T, C, O)
    if key not in _NC_CACHE:
        _NC_CACHE[key] = build_nc(T, C, O)
    return _NC_CACHE[key]


def kernel(x, ve, qkv_w, lambdas, c_proj_w):
    x = np.asarray(x, np.float32)
    ve = np.asarray(ve, np.float32)
    qkv_w = np.asarray(qkv_w, np.float32)
    lambdas = np.asarray(lambdas, np.float32)
    c_proj_w = np.asarray(c_proj_w, np.float32)
    B, T, C = x.shape
    O = C // 2
    nc = get_nc(T, C, O)
    in_maps = [prep_core_inputs(x, ve, qkv_w, lambdas, c_proj_w, c)
               for c in range(N_CORES)]
    res = run_bass_kernel_spmd(nc, in_maps, list(range(N_CORES)))
    out = np.stack([res.results[2 * b]["out"] for b in range(B)])
    return out


def measure_hw_time_ns(inputs, r1=8, r2=2008, runs=4):
    """Slope-timing: in-NEFF For_i repetition, min-wall over runs."""
    import time as _time
    x = np.asarray(inputs["x"], np.float32)
    B, T, C = x.shape
    O = C // 2
    in_maps = [prep_core_inputs(x, np.asarray(inputs["ve"], np.float32),
                                np.asarray(inputs["qkv_w"], np.float32),
                                np.asarray(inputs["lambdas"], np.float32),
                                np.asarray(inputs["c_proj_w"], np.float32), c)
               for c in range(N_CORES)]
    times = {}
    for reps in (r1, r2):
        nc = build_nc(T, C, O, loop_reps=reps)
        best = float("inf")
        for _ in range(runs):
            t0 = _time.time()
            run_bass_kernel_spmd(nc, in_maps, list(range(N_CORES)))
            best = min(best, _time.time() - t0)
        times[reps] = best
    return (times[r2] - times[r1]) / (r2 - r1) * 1e9

